# revision 35
# baseline (speedup 1.0000x reference)
"""Trainium2 Bass kernel for nn_Attention_43516608643501.

Cross-attention: Q = out_d [T,B,H]; K = V = sum of fwd/bwd halves of out_e
-> [S,B,H]; scores = Q @ K^T per batch (contraction over H, no scaling);
softmax over the source dim S; context = P @ V -> output [T,B,H].

Sharding: data-parallel over batch (dim 1): 2 batches per core x 8 cores,
no cross-core communication.

Layout: scores are computed in [t_partition, s_free] tiles so the softmax
max and sum are free-dim reductions (DVE reduce_max + the ACT activation's
accum_out register). The per-row max makes the kernel robust to any input
realization (exp args <= 0, P in [0,1], l in [1,S]) and lets P live in
fp16. P is then transposed back to [s,t] blocks on the PE (fp16 transpose,
1 cyc/row) for the P^T @ V accumulation.

Numerics: both matmuls run in fp16 (full PE rate; fp16's 4.9e-4 rounding
vs bf16's 4e-3 matters because the scores carry no 1/sqrt(H) scaling, so
near-ties in the softmax amplify score error by exp()).
"""

import numpy as np
from contextlib import ExitStack

S, T, B, H = 2048, 2048, 16, 512
NCORES = 8
BLOC = B // NCORES  # batches per core
P128 = 128
NS = S // P128  # 16 s-tiles
NT = T // P128  # 16 t-tiles
NH = H // P128  # 4 h-chunks of the contraction
SC = 512  # s-chunk width (scores tile columns)
NSC = S // SC  # 4 s-chunks per t-tile

_cached_nc = None


def _build():
    import concourse.bacc as bacc
    import concourse.tile as tile
    from concourse import mybir
    from concourse.masks import make_identity

    f32 = mybir.dt.float32
    f16 = mybir.dt.float16

    nc = bacc.Bacc(None, target_bir_lowering=False)
    d_oe = nc.dram_tensor("out_e", [S, BLOC, 2 * H], f32, kind="ExternalInput")
    d_od = nc.dram_tensor("out_d", [T, BLOC, H], f32, kind="ExternalInput")
    d_out = nc.dram_tensor("out", [T, BLOC, H], f32, kind="ExternalOutput")

    with ExitStack() as ctx:
        tc = ctx.enter_context(tile.TileContext(nc))
        singles = ctx.enter_context(tc.tile_pool(name="singles", bufs=1))
        loads = ctx.enter_context(tc.tile_pool(name="loads", bufs=10))
        persist = ctx.enter_context(tc.tile_pool(name="persist", bufs=2))
        ptile = ctx.enter_context(tc.tile_pool(name="ptile", bufs=2))
        outs = ctx.enter_context(tc.tile_pool(name="outs", bufs=3))
        small = ctx.enter_context(tc.tile_pool(name="small", bufs=3))
        # PSUM: 8 banks = ps_s0..3 (4) + ptr (2) + ps_c (2)
        ps_s_pool = ctx.enter_context(tc.tile_pool(name="ps_s_pool", bufs=1, space="PSUM"))
        ps_tr = ctx.enter_context(tc.tile_pool(name="ps_tr", bufs=2, space="PSUM"))
        ps_cp = ctx.enter_context(tc.tile_pool(name="ps_cp", bufs=2, space="PSUM"))

        id16 = singles.tile([P128, P128], f16)
        make_identity(nc, id16)

        for b in range(BLOC):
            # ---- prep: oe halves summed to fp16 (V and transpose source);
            # oeT/odT = h-on-partition layouts for the scores matmul.
            # Interleave oe/od tiles and keep the transposed tensors
            # chunk-granular so the first scores matmul only depends on the
            # first few loads, not on the whole prep phase. ----
            oe_nat = []
            oeT_c = [
                persist.tile([P128, NH, SC], f16, tag=f"oeT{ci}", name=f"oeT{ci}")
                for ci in range(NSC)
            ]
            odT_t = [
                persist.tile([P128, NH, P128], f16, tag=f"odT{tt}", name=f"odT{tt}")
                for tt in range(NT)
            ]
            def prep_oe(k):
                raw = loads.tile([P128, 2 * H], f32, tag="raw", name="raw")
                nc.sync.dma_start(
                    out=raw, in_=d_oe[k * P128:(k + 1) * P128, b, :]
                )
                nat = persist.tile(
                    [P128, H], f16, tag=f"oenat{k}", name=f"oenat{k}"
                )
                nc.vector.tensor_add(nat, raw[:, 0:H], raw[:, H:2 * H])
                oe_nat.append(nat)
                trp = ps_tr.tile([P128, H], f16, tag="tr", name="tr_oe")
                for hc in range(NH):
                    nc.tensor.transpose(
                        trp[:, hc * P128:(hc + 1) * P128],
                        nat[:, hc * P128:(hc + 1) * P128],
                        id16,
                    )
                dst = oeT_c[k // 4][:, :, (k % 4) * P128:(k % 4 + 1) * P128]
                src = trp.rearrange("p (h s) -> p h s", h=NH)
                if k % 2 == 0:
                    nc.scalar.copy(dst, src)
                else:
                    nc.vector.tensor_copy(dst, src)

            def prep_od(k):
                odr = loads.tile([P128, H], f32, tag="odr", name="odr")
                nc.sync.dma_start(
                    out=odr, in_=d_od[k * P128:(k + 1) * P128, b, :]
                )
                odf = loads.tile([P128, H], f16, tag="odf", name="odf")
                nc.vector.tensor_copy(odf, odr)
                trp2 = ps_tr.tile([P128, H], f16, tag="tr", name="tr_od")
                for hc in range(NH):
                    nc.tensor.transpose(
                        trp2[:, hc * P128:(hc + 1) * P128],
                        odf[:, hc * P128:(hc + 1) * P128],
                        id16,
                    )
                dst2 = odT_t[k][:, :, :]
                src2 = trp2.rearrange("p (h t) -> p h t", h=NH)
                if k % 2 == 0:
                    nc.vector.tensor_copy(dst2, src2)
                else:
                    nc.scalar.copy(dst2, src2)



            # ---- main: per t-tile of 128 query rows, software-pipelined:
            # stage 1 (tile tt): scores matmuls + max + exp(P);
            # stage 2 (tile tt-1): P transposes, PSUM->SBUF copies, P^T @ V.
            # PE alternates mm1(tt) / tr+mm2(tt-1) so the softmax
            # (DVE reductions + ACT exp) of tt hides under PE work. ----
            def stage1_begin(tt):
                mx = small.tile([P128, NSC], f32, tag="mx", name="mx")
                return {"tt": tt, "mx": mx, "ps_s": []}

            def stage1_chunk(st1, ci):
                tt, mx = st1["tt"], st1["mx"]
                pss = ps_s_pool.tile(
                    [P128, SC], f32, tag=f"ps_s{ci}", name=f"ps_s{ci}"
                )
                for hc in range(NH):
                    nc.tensor.matmul(
                        pss,
                        odT_t[tt][:, hc, :],
                        oeT_c[ci][:, hc, :],
                        start=(hc == 0),
                        stop=(hc == NH - 1),
                    )
                nc.vector.reduce_max(
                    mx[:, ci:ci + 1], pss, axis=mybir.AxisListType.X
                )
                st1["ps_s"].append(pss)

            def stage1_finish(st1):
                tt, mx, ps_s = st1["tt"], st1["mx"], st1["ps_s"]
                neg_m = small.tile([P128, 1], f32, tag="neg_m", name="neg_m")
                m = small.tile([P128, 1], f32, tag="m", name="m")
                nc.vector.reduce_max(m, mx, axis=mybir.AxisListType.X)
                nc.vector.tensor_scalar_mul(neg_m, m, -1.0)

                lacc = small.tile([P128, NSC], f32, tag="lacc", name="lacc")
                pts = []
                for ci in range(NSC):
                    pt = ptile.tile([P128, SC], f16, tag=f"pt{ci}", name=f"pt{ci}")
                    nc.scalar.activation(
                        pt, ps_s[ci], mybir.ActivationFunctionType.Exp,
                        bias=neg_m, scale=1.0,
                        accum_out=lacc[:, ci:ci + 1],
                    )
                    pts.append(pt)
                l = small.tile([P128, 1], f32, tag="l", name="l")
                nc.vector.reduce_sum(l, lacc, axis=mybir.AxisListType.X)
                linv = small.tile([P128, 1], f32, tag="linv", name="linv")
                nc.vector.reciprocal(linv, l)
                return tt, pts, linv

            def stage1(tt):
                st1 = stage1_begin(tt)
                for ci in range(NSC):
                    stage1_chunk(st1, ci)
                return stage1_finish(st1)

            def stage2(state):
                tt, pts, linv = state
                tb = slice(tt * P128, (tt + 1) * P128)
                pT_c = []
                ptr = None
                for ci in range(NSC):
                    if ci % 2 == 0:
                        ptr = ps_tr.tile([P128, 2, SC], f16, tag="tr", name="ptr")
                    half = ci % 2
                    for j in range(SC // P128):
                        nc.tensor.transpose(
                            ptr[:, half, j * P128:(j + 1) * P128],
                            pts[ci][:, j * P128:(j + 1) * P128],
                            id16,
                        )
                    pc = ptile.tile([P128, SC], f16, tag=f"pT{ci}", name=f"pT{ci}")
                    if ci < 2:
                        nc.scalar.copy(pc, ptr[:, half, :])
                    else:
                        nc.vector.tensor_copy(pc, ptr[:, half, :])
                    pT_c.append(pc)

                ps_c = ps_cp.tile([P128, H], f32, tag="ps_c", name="ps_c")
                for k in range(NS):
                    nc.tensor.matmul(
                        ps_c,
                        pT_c[k // 4][:, (k % 4) * P128:(k % 4 + 1) * P128],
                        oe_nat[k],
                        start=(k == 0), stop=(k == NS - 1),
                    )
                ot = outs.tile([P128, H], f32, tag="ot", name="ot")
                nc.scalar.activation(
                    ot, ps_c, mybir.ActivationFunctionType.Identity,
                    bias=0.0, scale=linv,
                )
                nc.sync.dma_start(out=d_out[tb, b, :], in_=ot)

            # interleave the first t-tile's scores chunks into the oe prep:
            # chunk ci only needs oe tiles 4ci..4ci+3, so the PE starts real
            # work while later oe tiles are still loading.
            st1_0 = stage1_begin(0)
            for g in range(NSC):
                for k in range(4 * g, 4 * g + 4):
                    prep_oe(k)
                if g == 0:
                    prep_od(0)
                stage1_chunk(st1_0, g)
            prep_od(1)

            prev = stage1_finish(st1_0)
            for tt in range(1, NT):
                if tt + 1 < NT:
                    prep_od(tt + 1)
                state = stage1(tt)
                stage2(prev)
                prev = state
            stage2(prev)

    nc.finalize()
    return nc


def _ensure_devices():
    """Make sure the 8 NeuronCores are visible to jax.devices().

    The calling harness may have pinned jax to cpu (JAX_PLATFORMS=cpu is a
    common pin for running the jax reference); the Bass SPMD launcher uses
    jax.devices(), so re-point jax at the neuron platform if needed.
    """
    import os
    import jax

    try:
        devs = jax.devices()
    except Exception:
        devs = []
    if sum(1 for d in devs if d.platform != "cpu") >= NCORES:
        return
    for plats in ("axon,cpu", None):
        try:
            if plats is None:
                os.environ.pop("JAX_PLATFORMS", None)
            else:
                os.environ["JAX_PLATFORMS"] = plats
            jax.config.update("jax_platforms", plats)
            from jax.extend.backend import clear_backends

            clear_backends()
            devs = jax.devices()
            if sum(1 for d in devs if d.platform != "cpu") >= NCORES:
                return
        except Exception:
            continue


def kernel(in_e=None, out_e=None, out_d=None, **kwargs):
    global _cached_nc
    from concourse.bass_utils import run_bass_kernel_spmd

    _ensure_devices()

    out_e = np.asarray(out_e, dtype=np.float32)
    out_d = np.asarray(out_d, dtype=np.float32)
    if _cached_nc is None:
        _cached_nc = _build()
    in_maps = []
    for c in range(NCORES):
        bsl = slice(c * BLOC, (c + 1) * BLOC)
        in_maps.append({
            "out_e": np.ascontiguousarray(out_e[:, bsl, :]),
            "out_d": np.ascontiguousarray(out_d[:, bsl, :]),
        })
    res = run_bass_kernel_spmd(_cached_nc, in_maps, list(range(NCORES)))
    return np.concatenate([res.results[c]["out"] for c in range(NCORES)], axis=1)


# revision 36
# speedup vs baseline: 1.0000x; 1.0000x over previous
"""Trainium2 Bass kernel for nn_Attention_43516608643501.

Cross-attention: Q = out_d [T,B,H]; K = V = sum of fwd/bwd halves of out_e
-> [S,B,H]; scores = Q @ K^T per batch (contraction over H, no scaling);
softmax over the source dim S; context = P @ V -> output [T,B,H].

Sharding: data-parallel over batch (dim 1): 2 batches per core x 8 cores,
no cross-core communication.

Layout: scores are computed in [t_partition, s_free] tiles so the softmax
max and sum are free-dim reductions (DVE reduce_max + the ACT activation's
accum_out register). The per-row max makes the kernel robust to any input
realization (exp args <= 0, P in [0,1], l in [1,S]) and lets P live in
fp16. P is then transposed back to [s,t] blocks on the PE (fp16 transpose,
1 cyc/row) for the P^T @ V accumulation.

Numerics: both matmuls run in fp16 (full PE rate; fp16's 4.9e-4 rounding
vs bf16's 4e-3 matters because the scores carry no 1/sqrt(H) scaling, so
near-ties in the softmax amplify score error by exp()).
"""

import numpy as np
from contextlib import ExitStack

S, T, B, H = 2048, 2048, 16, 512
NCORES = 8
BLOC = B // NCORES  # batches per core
P128 = 128
NS = S // P128  # 16 s-tiles
NT = T // P128  # 16 t-tiles
NH = H // P128  # 4 h-chunks of the contraction
SC = 512  # s-chunk width (scores tile columns)
NSC = S // SC  # 4 s-chunks per t-tile

_cached_nc = None


def _build():
    import concourse.bacc as bacc
    import concourse.tile as tile
    from concourse import mybir
    from concourse.masks import make_identity

    f32 = mybir.dt.float32
    f16 = mybir.dt.float16

    nc = bacc.Bacc(None, target_bir_lowering=False)
    d_oe = nc.dram_tensor("out_e", [S, BLOC, 2 * H], f32, kind="ExternalInput")
    d_od = nc.dram_tensor("out_d", [T, BLOC, H], f32, kind="ExternalInput")
    d_out = nc.dram_tensor("out", [T, BLOC, H], f32, kind="ExternalOutput")

    with ExitStack() as ctx:
        tc = ctx.enter_context(tile.TileContext(nc))
        singles = ctx.enter_context(tc.tile_pool(name="singles", bufs=1))
        loads = ctx.enter_context(tc.tile_pool(name="loads", bufs=9))
        ptp = ctx.enter_context(tc.tile_pool(name="ptp", bufs=3))
        persist = ctx.enter_context(tc.tile_pool(name="persist", bufs=2))
        ptile = ctx.enter_context(tc.tile_pool(name="ptile", bufs=2))
        outs = ctx.enter_context(tc.tile_pool(name="outs", bufs=3))
        small = ctx.enter_context(tc.tile_pool(name="small", bufs=3))
        # PSUM: 8 banks = ps_s0..3 (4) + ptr (2) + ps_c (2)
        ps_s_pool = ctx.enter_context(tc.tile_pool(name="ps_s_pool", bufs=1, space="PSUM"))
        ps_tr = ctx.enter_context(tc.tile_pool(name="ps_tr", bufs=2, space="PSUM"))
        ps_cp = ctx.enter_context(tc.tile_pool(name="ps_cp", bufs=2, space="PSUM"))

        id16 = singles.tile([P128, P128], f16)
        make_identity(nc, id16)

        for b in range(BLOC):
            # ---- prep: oe halves summed to fp16 (V and transpose source);
            # oeT/odT = h-on-partition layouts for the scores matmul.
            # Interleave oe/od tiles and keep the transposed tensors
            # chunk-granular so the first scores matmul only depends on the
            # first few loads, not on the whole prep phase. ----
            oe_nat = []
            oeT_c = [
                persist.tile([P128, NH, SC], f16, tag=f"oeT{ci}", name=f"oeT{ci}")
                for ci in range(NSC)
            ]
            odT_t = [
                persist.tile([P128, NH, P128], f16, tag=f"odT{tt}", name=f"odT{tt}")
                for tt in range(NT)
            ]
            def prep_oe(k):
                raw = loads.tile([P128, 2 * H], f32, tag="raw", name="raw")
                nc.sync.dma_start(
                    out=raw, in_=d_oe[k * P128:(k + 1) * P128, b, :]
                )
                nat = persist.tile(
                    [P128, H], f16, tag=f"oenat{k}", name=f"oenat{k}"
                )
                nc.vector.tensor_add(nat, raw[:, 0:H], raw[:, H:2 * H])
                oe_nat.append(nat)
                trp = ps_tr.tile([P128, H], f16, tag="tr", name="tr_oe")
                for hc in range(NH):
                    nc.tensor.transpose(
                        trp[:, hc * P128:(hc + 1) * P128],
                        nat[:, hc * P128:(hc + 1) * P128],
                        id16,
                    )
                dst = oeT_c[k // 4][:, :, (k % 4) * P128:(k % 4 + 1) * P128]
                src = trp.rearrange("p (h s) -> p h s", h=NH)
                if k % 2 == 0:
                    nc.scalar.copy(dst, src)
                else:
                    nc.vector.tensor_copy(dst, src)

            def prep_od(k):
                odr = loads.tile([P128, H], f32, tag="odr", name="odr")
                nc.sync.dma_start(
                    out=odr, in_=d_od[k * P128:(k + 1) * P128, b, :]
                )
                odf = loads.tile([P128, H], f16, tag="odf", name="odf")
                nc.vector.tensor_copy(odf, odr)
                trp2 = ps_tr.tile([P128, H], f16, tag="tr", name="tr_od")
                for hc in range(NH):
                    nc.tensor.transpose(
                        trp2[:, hc * P128:(hc + 1) * P128],
                        odf[:, hc * P128:(hc + 1) * P128],
                        id16,
                    )
                dst2 = odT_t[k][:, :, :]
                src2 = trp2.rearrange("p (h t) -> p h t", h=NH)
                if k % 2 == 0:
                    nc.vector.tensor_copy(dst2, src2)
                else:
                    nc.scalar.copy(dst2, src2)



            # ---- main: per t-tile of 128 query rows, software-pipelined:
            # stage 1 (tile tt): scores matmuls + max + exp(P);
            # stage 2 (tile tt-1): P transposes, PSUM->SBUF copies, P^T @ V.
            # PE alternates mm1(tt) / tr+mm2(tt-1) so the softmax
            # (DVE reductions + ACT exp) of tt hides under PE work. ----
            def stage1_begin(tt):
                mx = small.tile([P128, NSC], f32, tag="mx", name="mx")
                return {"tt": tt, "mx": mx, "ps_s": []}

            def stage1_chunk(st1, ci):
                tt, mx = st1["tt"], st1["mx"]
                pss = ps_s_pool.tile(
                    [P128, SC], f32, tag=f"ps_s{ci}", name=f"ps_s{ci}"
                )
                for hc in range(NH):
                    nc.tensor.matmul(
                        pss,
                        odT_t[tt][:, hc, :],
                        oeT_c[ci][:, hc, :],
                        start=(hc == 0),
                        stop=(hc == NH - 1),
                    )
                nc.vector.reduce_max(
                    mx[:, ci:ci + 1], pss, axis=mybir.AxisListType.X
                )
                st1["ps_s"].append(pss)

            def stage1_finish(st1):
                tt, mx, ps_s = st1["tt"], st1["mx"], st1["ps_s"]
                neg_m = small.tile([P128, 1], f32, tag="neg_m", name="neg_m")
                m = small.tile([P128, 1], f32, tag="m", name="m")
                nc.vector.reduce_max(m, mx, axis=mybir.AxisListType.X)
                nc.vector.tensor_scalar_mul(neg_m, m, -1.0)

                lacc = small.tile([P128, NSC], f32, tag="lacc", name="lacc")
                pts = []
                for ci in range(NSC):
                    pt = ptp.tile([P128, SC], f16, tag=f"pt{ci}", name=f"pt{ci}")
                    nc.scalar.activation(
                        pt, ps_s[ci], mybir.ActivationFunctionType.Exp,
                        bias=neg_m, scale=1.0,
                        accum_out=lacc[:, ci:ci + 1],
                    )
                    pts.append(pt)
                l = small.tile([P128, 1], f32, tag="l", name="l")
                nc.vector.reduce_sum(l, lacc, axis=mybir.AxisListType.X)
                linv = small.tile([P128, 1], f32, tag="linv", name="linv")
                nc.vector.reciprocal(linv, l)
                return tt, pts, linv

            def stage1(tt):
                st1 = stage1_begin(tt)
                for ci in range(NSC):
                    stage1_chunk(st1, ci)
                return stage1_finish(st1)

            def stage2(state):
                tt, pts, linv = state
                tb = slice(tt * P128, (tt + 1) * P128)
                pT_c = []
                ptr = None
                for ci in range(NSC):
                    if ci % 2 == 0:
                        ptr = ps_tr.tile([P128, 2, SC], f16, tag="tr", name="ptr")
                    half = ci % 2
                    for j in range(SC // P128):
                        nc.tensor.transpose(
                            ptr[:, half, j * P128:(j + 1) * P128],
                            pts[ci][:, j * P128:(j + 1) * P128],
                            id16,
                        )
                    pc = ptile.tile([P128, SC], f16, tag=f"pT{ci}", name=f"pT{ci}")
                    if ci < 2:
                        nc.scalar.copy(pc, ptr[:, half, :])
                    else:
                        nc.vector.tensor_copy(pc, ptr[:, half, :])
                    pT_c.append(pc)

                ps_c = ps_cp.tile([P128, H], f32, tag="ps_c", name="ps_c")
                for k in range(NS):
                    nc.tensor.matmul(
                        ps_c,
                        pT_c[k // 4][:, (k % 4) * P128:(k % 4 + 1) * P128],
                        oe_nat[k],
                        start=(k == 0), stop=(k == NS - 1),
                    )
                ot = outs.tile([P128, H], f32, tag="ot", name="ot")
                nc.scalar.activation(
                    ot, ps_c, mybir.ActivationFunctionType.Identity,
                    bias=0.0, scale=linv,
                )
                nc.sync.dma_start(out=d_out[tb, b, :], in_=ot)

            # interleave the first t-tile's scores chunks into the oe prep:
            # chunk ci only needs oe tiles 4ci..4ci+3, so the PE starts real
            # work while later oe tiles are still loading.
            st1_0 = stage1_begin(0)
            for g in range(NSC):
                for k in range(4 * g, 4 * g + 4):
                    prep_oe(k)
                if g == 0:
                    prep_od(0)
                stage1_chunk(st1_0, g)
            prep_od(1)

            prev = stage1_finish(st1_0)
            for tt in range(1, NT):
                if tt + 1 < NT:
                    prep_od(tt + 1)
                state = stage1(tt)
                stage2(prev)
                prev = state
            stage2(prev)

    nc.finalize()
    return nc


def _ensure_devices():
    """Make sure the 8 NeuronCores are visible to jax.devices().

    The calling harness may have pinned jax to cpu (JAX_PLATFORMS=cpu is a
    common pin for running the jax reference); the Bass SPMD launcher uses
    jax.devices(), so re-point jax at the neuron platform if needed.
    """
    import os
    import jax

    try:
        devs = jax.devices()
    except Exception:
        devs = []
    if sum(1 for d in devs if d.platform != "cpu") >= NCORES:
        return
    for plats in ("axon,cpu", None):
        try:
            if plats is None:
                os.environ.pop("JAX_PLATFORMS", None)
            else:
                os.environ["JAX_PLATFORMS"] = plats
            jax.config.update("jax_platforms", plats)
            from jax.extend.backend import clear_backends

            clear_backends()
            devs = jax.devices()
            if sum(1 for d in devs if d.platform != "cpu") >= NCORES:
                return
        except Exception:
            continue


def kernel(in_e=None, out_e=None, out_d=None, **kwargs):
    global _cached_nc
    from concourse.bass_utils import run_bass_kernel_spmd

    _ensure_devices()

    out_e = np.asarray(out_e, dtype=np.float32)
    out_d = np.asarray(out_d, dtype=np.float32)
    if _cached_nc is None:
        _cached_nc = _build()
    in_maps = []
    for c in range(NCORES):
        bsl = slice(c * BLOC, (c + 1) * BLOC)
        in_maps.append({
            "out_e": np.ascontiguousarray(out_e[:, bsl, :]),
            "out_d": np.ascontiguousarray(out_d[:, bsl, :]),
        })
    res = run_bass_kernel_spmd(_cached_nc, in_maps, list(range(NCORES)))
    return np.concatenate([res.results[c]["out"] for c in range(NCORES)], axis=1)
